# revision 23
# baseline (speedup 1.0000x reference)
"""Trainium2 Bass kernel for nn_CNNVectorForm (LeNet-style CNN, batch 8192).

Pipeline per core (data-parallel over batch, 1024 images/core):
  conv 5x5 VALID (1->20ch, 28->24)  -> 2x2 maxpool -> fc1(2880->500) + relu
  -> fc2(500->10) + softmax

Device formulation:
  * All activations feature-major [features, batch] so the PE contracts
    along partitions; batch rides the free dim (512 per tile).
  * Conv as a Toeplitz matmul: for each output row r and 12-wide column
    block, one K=80 (5 rows x 16 cols of input) x M=120 matmul produces
    [20ch x 12cols, batch].  Output columns are split into even/odd
    M-tiles so the 2x2 maxpool is three partition-aligned tensor_max ops.
  * fc1 weights are host-permuted to the pooled-feature order, so fc1 is
    24 accumulating K=120 matmuls per 125-neuron M-tile.
  * conv bias is folded into the fc1 bias on the host (maxpool commutes
    with the per-channel constant).
  * fc2 runs batch-major (stationary operand = activations) so softmax
    reduces along the free dim; fc2 bias via a K=1 ones matmul.
  * Matmuls use float32r (1 cycle/row at N>=256 vs 4 for fp32).
"""

import numpy as np

N, H, W = 8192, 28, 28
COUT, KS = 20, 5
NCORES = 8
NPC = N // NCORES  # images per core
CONV_W_OUT = 24
PH = 12            # pooled rows
FC1_IN, FC1_OUT, FC2_OUT = 2880, 500, 10
MT, MTS = 4, 125   # fc1 M tiles
KB, KBS = 24, 120  # a1 feature blocks (one per (pooled row, column half))

_cache = {}


def _build(npc, nb):
    from contextlib import ExitStack

    import concourse.tile as tile
    from concourse import bacc, mybir

    f32 = mybir.dt.float32
    f32r = mybir.dt.float32r
    nbt = npc // nb

    nc = bacc.Bacc(
        "TRN2",
        target_bir_lowering=False,
        debug=False,
        enable_asserts=False,
        num_devices=NCORES,
    )

    # host-im2col'd input: xg[jb, p, r, b] = x[(r + p//16)*28 + 12*jb + p%16, b]
    xg_d = nc.dram_tensor(
        "xg", [2, 80, CONV_W_OUT, npc], f32r, kind="ExternalInput"
    ).ap()
    t_d = nc.dram_tensor("tmat", [80, 240], f32r, kind="ExternalInput").ap()
    w1_d = nc.dram_tensor("w1", [KB, KBS, FC1_OUT], f32r, kind="ExternalInput").ap()
    b1_d = nc.dram_tensor("b1", [MTS, MT], f32, kind="ExternalInput").ap()
    w2_d = nc.dram_tensor("w2", [MTS, MT * FC2_OUT], f32r, kind="ExternalInput").ap()
    b2_d = nc.dram_tensor("b2", [FC2_OUT, 1], f32, kind="ExternalInput").ap()
    o_d = nc.dram_tensor("out", [npc, FC2_OUT], f32, kind="ExternalOutput").ap()

    with tile.TileContext(nc) as tc, ExitStack() as ctx:
        const = ctx.enter_context(tc.tile_pool(name="const", bufs=1))
        w1pool = ctx.enter_context(tc.tile_pool(name="w1", bufs=KB))
        gpool = ctx.enter_context(tc.tile_pool(name="gather", bufs=8))
        a1pool = ctx.enter_context(tc.tile_pool(name="a1", bufs=8))
        tmppool = ctx.enter_context(tc.tile_pool(name="ptmp", bufs=4))
        a2pool = ctx.enter_context(tc.tile_pool(name="a2", bufs=2 * MT))
        smpool = ctx.enter_context(tc.tile_pool(name="softmax", bufs=4))
        cpsum = ctx.enter_context(tc.tile_pool(name="cpsum", bufs=4, space="PSUM"))
        fpsum = ctx.enter_context(tc.tile_pool(name="fpsum", bufs=4, space="PSUM"))

        from concourse.masks import make_identity

        t240 = const.tile([80, 240], f32r)
        nc.sync.dma_start(t240[:], t_d[:])
        b1t = const.tile([MTS, MT], f32)
        nc.gpsimd.dma_start(b1t[:], b1_d[:])
        w2t = const.tile([MTS, MT * FC2_OUT], f32r)
        nc.gpsimd.dma_start(w2t[:], w2_d[:])
        b2t = const.tile([FC2_OUT, 1], f32)
        nc.gpsimd.dma_start(b2t[:], b2_d[:])
        ident = const.tile([FC2_OUT, FC2_OUT], f32)
        make_identity(nc, ident[:])
        # w1 tiles are DMA'd lazily inside the first batch's conv loop on the
        # (otherwise idle) gpsimd SWDGE queue so neither the sync HWDGE ring
        # nor the scalar engine pay for the 5.8 MB of weight traffic.
        w1t = [None] * KB

        for bt in range(nbt):
            b0 = bt * nb
            a1 = [None] * KB
            # fc1 accumulators for all 4 M-tiles ride along with the conv
            # loop, skewed by 2 blocks: 4 dependency-free fc1 matmuls per
            # quad keep the PE gap-free so HAM stays at full clock.
            fp = [
                fpsum.tile([MTS, nb], f32, tag="fps", name=f"fp{bt}_{mt}")
                for mt in range(MT)
            ]
            for kb in range(KB + 2):
                if kb >= 2:
                    j = kb - 2
                    for mt in range(MT):
                        nc.tensor.matmul(
                            fp[mt][:],
                            w1t[j][:, mt * MTS : (mt + 1) * MTS],
                            a1[j][:],
                            start=(j == 0),
                            stop=(j == KB - 1),
                        )
                if kb >= KB:
                    continue
                ip, jb = kb // 2, kb % 2
                g = []
                for dr in range(2):
                    gt = gpool.tile([80, nb], f32r, tag="g")
                    r = 2 * ip + dr
                    nc.sync.dma_start(gt[:], xg_d[jb, :, r, b0 : b0 + nb])
                    g.append(gt)
                if bt == 0 and w1t[kb] is None:
                    t = w1pool.tile([KBS, FC1_OUT], f32r, tag="w1")
                    nc.gpsimd.dma_start(t[:], w1_d[kb])
                    w1t[kb] = t
                # even-column psums in two single-bank tiles (evicted early by
                # ACT), odd columns in one double-bank tile so the DVE
                # even/odd max is a single wide op.
                pe0 = cpsum.tile([KBS, nb], f32, tag="pe", bufs=2,
                                 name=f"pe0_{kb}")
                pe1 = cpsum.tile([KBS, nb], f32, tag="pe", bufs=2,
                                 name=f"pe1_{kb}")
                pod = cpsum.tile([KBS, 2 * nb], f32, tag="po", bufs=1,
                                 name=f"pod_{kb}")
                nc.tensor.matmul(
                    pe0[:], t240[:, 0:120], g[0][:], start=True, stop=True)
                nc.tensor.matmul(
                    pe1[:], t240[:, 0:120], g[1][:], start=True, stop=True)
                nc.tensor.matmul(
                    pod[:, 0:nb], t240[:, 120:240], g[0][:],
                    start=True, stop=True)
                nc.tensor.matmul(
                    pod[:, nb : 2 * nb], t240[:, 120:240], g[1][:],
                    start=True, stop=True)
                sp = tmppool.tile([KBS, 2 * nb], f32, tag="s")
                nc.scalar.copy(sp[:, 0:nb], pe0[:])
                nc.scalar.copy(sp[:, nb : 2 * nb], pe1[:])
                m = tmppool.tile([KBS, 2 * nb], f32, tag="m")
                nc.vector.tensor_max(m[:], sp[:], pod[:])
                ab = a1pool.tile([KBS, nb], f32r, tag="a1")
                nc.vector.tensor_max(ab[:], m[:, 0:nb], m[:, nb : 2 * nb])
                a1[kb] = ab

            a2t = [None] * MT
            for mt in range(MT):
                a2 = a2pool.tile([MTS, nb], f32r, tag="a2")
                nc.scalar.activation(
                    a2[:],
                    fp[mt][:],
                    mybir.ActivationFunctionType.Relu,
                    bias=b1t[:, mt : mt + 1],
                )
                a2t[mt] = a2

            # fc2 feature-major: weights stationary, batch streams; softmax
            # needs batch on partitions, so PE-transpose 128-wide slices.
            p2f = fpsum.tile([FC2_OUT, nb], f32, tag="fps", name=f"p2f_{bt}")
            for mt in range(MT):
                nc.tensor.matmul(
                    p2f[:],
                    w2t[:, mt * FC2_OUT : (mt + 1) * FC2_OUT],
                    a2t[mt][:],
                    start=(mt == 0),
                    stop=(mt == MT - 1),
                )
            s2 = smpool.tile([FC2_OUT, nb], f32, tag="s2")
            nc.scalar.activation(
                s2[:], p2f[:], mybir.ActivationFunctionType.Identity,
                bias=b2t[:, 0:1],
            )
            sub = min(128, nb)
            for s in range(nb // sub):
                tp = fpsum.tile([sub, FC2_OUT], f32, tag="fps",
                                name=f"tp_{bt}_{s}")
                nc.tensor.transpose(
                    tp[:], s2[:, s * sub : (s + 1) * sub], ident[:]
                )
                e = smpool.tile([sub, FC2_OUT], f32, tag="e")
                ssum = smpool.tile([sub, 1], f32, tag="ss")
                nc.scalar.activation(
                    e[:], tp[:], mybir.ActivationFunctionType.Exp,
                    accum_out=ssum[:],
                )
                rinv = smpool.tile([sub, 1], f32, tag="ri")
                nc.vector.reciprocal(rinv[:], ssum[:])
                ot = smpool.tile([sub, FC2_OUT], f32, tag="ot")
                nc.vector.tensor_scalar_mul(ot[:], e[:], rinv[:])
                nc.sync.dma_start(o_d[b0 + s * sub : b0 + (s + 1) * sub, :], ot[:])

    nc.compile()
    return nc


def _prep_weights(conv_w, conv_b, fc1_w, fc1_b, fc2_w, fc2_b):
    conv_w = np.asarray(conv_w, np.float32).reshape(COUT, KS, KS)
    conv_b = np.asarray(conv_b, np.float32)
    fc1_w = np.asarray(fc1_w, np.float32)
    fc1_b = np.asarray(fc1_b, np.float32)
    fc2_w = np.asarray(fc2_w, np.float32)
    fc2_b = np.asarray(fc2_b, np.float32)

    # Toeplitz conv matrix [80, 240]: row = di*16 + jjp (input row offset,
    # input col within 16-wide block); col m = eo*120 + c*6 + q for output
    # col jj = 2q + eo within the 12-wide block.
    T = np.zeros((80, 240), np.float32)
    for m in range(240):
        eo, c, q = m // 120, (m % 120) // 6, m % 6
        jj = 2 * q + eo
        for di in range(KS):
            for dj in range(KS):
                T[di * 16 + jj + dj, m] = conv_w[c, di, dj]

    # fc1 weights permuted to our pooled-feature order:
    # block kb = ip*2 + jb, within-block m = c*6 + q
    # -> original flat feature c*144 + ip*12 + jb*6 + q
    kbv = np.arange(KB)
    ipv, jbv = kbv // 2, kbv % 2
    ml = np.arange(KBS)
    cv, qv = ml // 6, ml % 6
    fidx = cv[None, :] * 144 + ipv[:, None] * 12 + jbv[:, None] * 6 + qv[None, :]
    w1 = np.ascontiguousarray(
        fc1_w.T[fidx.reshape(-1)].reshape(KB, KBS, FC1_OUT)
    )

    # conv bias folded into fc1 bias (pool-max commutes with per-channel const)
    cb_vec = np.repeat(conv_b, 144)
    b1p = fc1_b + fc1_w @ cb_vec
    b1 = np.ascontiguousarray(b1p.reshape(MT, MTS).T)

    w2 = np.ascontiguousarray(
        fc2_w.T.reshape(MT, MTS, FC2_OUT).transpose(1, 0, 2)
    ).reshape(MTS, MT * FC2_OUT)
    b2 = np.ascontiguousarray(fc2_b.reshape(FC2_OUT, 1))
    return T, w1, b1, w2, b2


# im2col pixel indices: idx[jb, di*16+jjp, r] = (r+di)*28 + 12*jb + jjp
_IDX = np.zeros((2, 80, CONV_W_OUT), np.int64)
for _jb in range(2):
    for _di in range(KS):
        for _jjp in range(16):
            for _r in range(CONV_W_OUT):
                _IDX[_jb, _di * 16 + _jjp, _r] = (_r + _di) * W + 12 * _jb + _jjp


def _prep_x(x_core):
    """x_core [784, npc] pixel-major -> xg [2, 80, 24, npc]."""
    return np.ascontiguousarray(x_core[_IDX.reshape(-1)].reshape(
        2, 80, CONV_W_OUT, x_core.shape[1]))


def _run(inputs, npc=NPC, nb=512, trace=False):
    from concourse import bass_utils

    key = (npc, nb)
    if key not in _cache:
        _cache[key] = _build(npc, nb)
    nc = _cache[key]

    T, w1, b1, w2, b2 = _prep_weights(
        inputs["conv_w"], inputs["conv_b"], inputs["fc1_w"],
        inputs["fc1_b"], inputs["fc2_w"], inputs["fc2_b"],
    )
    x = np.asarray(inputs["x"], np.float32).reshape(-1, H * W)
    n_total = x.shape[0]
    assert n_total == NCORES * npc
    xs = x.reshape(NCORES, npc, H * W).transpose(0, 2, 1)

    in_maps = [
        {"xg": _prep_x(xs[i]), "tmat": T, "w1": w1, "b1": b1, "w2": w2,
         "b2": b2}
        for i in range(NCORES)
    ]
    res = bass_utils.run_bass_kernel_spmd(
        nc, in_maps, core_ids=list(range(NCORES)), trace=trace
    )
    out = np.concatenate([res.results[i]["out"] for i in range(NCORES)], axis=0)
    return out, res


def kernel(**inputs):
    out, _ = _run(inputs)
    return out


# revision 24
# speedup vs baseline: 1.2670x; 1.2670x over previous
"""Trainium2 Bass kernel for nn_CNNVectorForm (LeNet-style CNN, batch 8192).

Pipeline per core (data-parallel over batch, 1024 images/core):
  conv 5x5 VALID (1->20ch, 28->24)  -> 2x2 maxpool -> fc1(2880->500) + relu
  -> fc2(500->10) + softmax

Device formulation:
  * All activations feature-major [features, batch] so the PE contracts
    along partitions; batch rides the free dim (512 per tile).
  * Conv as a Toeplitz matmul: for each output row r and 12-wide column
    block, one K=80 (5 rows x 16 cols of input) x M=120 matmul produces
    [20ch x 12cols, batch].  Output columns are split into even/odd
    M-tiles so the 2x2 maxpool is three partition-aligned tensor_max ops.
  * fc1 weights are host-permuted to the pooled-feature order, so fc1 is
    24 accumulating K=120 matmuls per 125-neuron M-tile.
  * conv bias is folded into the fc1 bias on the host (maxpool commutes
    with the per-channel constant).
  * fc2 runs batch-major (stationary operand = activations) so softmax
    reduces along the free dim; fc2 bias via a K=1 ones matmul.
  * Matmuls use float32r (1 cycle/row at N>=256 vs 4 for fp32).
"""

import numpy as np

N, H, W = 8192, 28, 28
COUT, KS = 20, 5
NCORES = 8
NPC = N // NCORES  # images per core
CONV_W_OUT = 24
PH = 12            # pooled rows
FC1_IN, FC1_OUT, FC2_OUT = 2880, 500, 10
MT, MTS = 4, 125   # fc1 M tiles
KB, KBS = 24, 120  # a1 feature blocks (one per (pooled row, column half))

_cache = {}


def _build(npc, nb):
    from contextlib import ExitStack

    import concourse.tile as tile
    from concourse import bacc, mybir

    f32 = mybir.dt.float32
    f32r = mybir.dt.float32r
    nbt = npc // nb

    nc = bacc.Bacc(
        "TRN2",
        target_bir_lowering=False,
        debug=False,
        enable_asserts=False,
        num_devices=NCORES,
    )

    # host-im2col'd input: xg[jb, p, r, b] = x[(r + p//16)*28 + 12*jb + p%16, b]
    xg_d = nc.dram_tensor(
        "xg", [2, 80, CONV_W_OUT, npc], f32r, kind="ExternalInput"
    ).ap()
    t_d = nc.dram_tensor("tmat", [80, 240], f32r, kind="ExternalInput").ap()
    w1_d = nc.dram_tensor("w1", [KB, KBS, FC1_OUT], f32r, kind="ExternalInput").ap()
    b1_d = nc.dram_tensor("b1", [MTS, MT], f32, kind="ExternalInput").ap()
    w2_d = nc.dram_tensor("w2", [MTS, MT * FC2_OUT], f32r, kind="ExternalInput").ap()
    b2_d = nc.dram_tensor("b2", [FC2_OUT, 1], f32, kind="ExternalInput").ap()
    o_d = nc.dram_tensor("out", [npc, FC2_OUT], f32, kind="ExternalOutput").ap()

    with tile.TileContext(nc) as tc, ExitStack() as ctx:
        const = ctx.enter_context(tc.tile_pool(name="const", bufs=1))
        w1pool = ctx.enter_context(tc.tile_pool(name="w1", bufs=KB))
        gpool = ctx.enter_context(tc.tile_pool(name="gather", bufs=8))
        a1pool = ctx.enter_context(tc.tile_pool(name="a1", bufs=8))
        tmppool = ctx.enter_context(tc.tile_pool(name="ptmp", bufs=4))
        a2pool = ctx.enter_context(tc.tile_pool(name="a2", bufs=2 * MT))
        smpool = ctx.enter_context(tc.tile_pool(name="softmax", bufs=4))
        cpsum = ctx.enter_context(tc.tile_pool(name="cpsum", bufs=4, space="PSUM"))
        fpsum = ctx.enter_context(tc.tile_pool(name="fpsum", bufs=4, space="PSUM"))

        from concourse.masks import make_identity

        t240 = const.tile([80, 240], f32r)
        nc.sync.dma_start(t240[:], t_d[:])
        b1t = const.tile([MTS, MT], f32)
        nc.gpsimd.dma_start(b1t[:], b1_d[:])
        w2t = const.tile([MTS, MT * FC2_OUT], f32r)
        nc.gpsimd.dma_start(w2t[:], w2_d[:])
        b2t = const.tile([FC2_OUT, 1], f32)
        nc.gpsimd.dma_start(b2t[:], b2_d[:])
        ident = const.tile([FC2_OUT, FC2_OUT], f32)
        make_identity(nc, ident[:])
        # w1 tiles are DMA'd lazily inside the first batch's conv loop on the
        # (otherwise idle) gpsimd SWDGE queue so neither the sync HWDGE ring
        # nor the scalar engine pay for the 5.8 MB of weight traffic.
        w1t = [None] * KB

        for bt in range(nbt):
            b0 = bt * nb
            a1 = [None] * KB
            # fc1 accumulators for all 4 M-tiles ride along with the conv
            # loop, skewed by 2 blocks: 4 dependency-free fc1 matmuls per
            # quad keep the PE gap-free so HAM stays at full clock.
            fp = [
                fpsum.tile([MTS, nb], f32, tag="fps", name=f"fp{bt}_{mt}")
                for mt in range(MT)
            ]
            for kb in range(KB + 2):
                if kb >= 2:
                    j = kb - 2
                    for mt in range(MT):
                        nc.tensor.matmul(
                            fp[mt][:],
                            w1t[j][:, mt * MTS : (mt + 1) * MTS],
                            a1[j][:],
                            start=(j == 0),
                            stop=(j == KB - 1),
                        )
                if kb >= KB:
                    continue
                ip, jb = kb // 2, kb % 2
                g = []
                for dr in range(2):
                    gt = gpool.tile([80, nb], f32r, tag="g")
                    r = 2 * ip + dr
                    nc.sync.dma_start(gt[:], xg_d[jb, :, r, b0 : b0 + nb])
                    g.append(gt)
                if bt == 0 and w1t[kb] is None:
                    t = w1pool.tile([KBS, FC1_OUT], f32r, tag="w1")
                    nc.gpsimd.dma_start(t[:], w1_d[kb])
                    w1t[kb] = t
                ps = [
                    cpsum.tile([KBS, nb], f32, tag="cps", name=f"cps{i}")
                    for i in range(4)
                ]
                for dr in range(2):
                    for eo in range(2):
                        nc.tensor.matmul(
                            ps[2 * dr + eo][:],
                            t240[:, 120 * eo : 120 * (eo + 1)],
                            g[dr][:],
                            start=True,
                            stop=True,
                        )
                s0 = tmppool.tile([KBS, nb], f32, tag="s")
                nc.scalar.copy(s0[:], ps[0][:])
                m0 = tmppool.tile([KBS, nb], f32, tag="m")
                nc.vector.tensor_max(m0[:], s0[:], ps[1][:])
                s1 = tmppool.tile([KBS, nb], f32, tag="s")
                nc.scalar.copy(s1[:], ps[2][:])
                m1 = tmppool.tile([KBS, nb], f32, tag="m")
                nc.vector.tensor_max(m1[:], s1[:], ps[3][:])
                ab = a1pool.tile([KBS, nb], f32r, tag="a1")
                nc.vector.tensor_max(ab[:], m0[:], m1[:])
                a1[kb] = ab

            a2t = [None] * MT
            for mt in range(MT):
                a2 = a2pool.tile([MTS, nb], f32r, tag="a2")
                nc.scalar.activation(
                    a2[:],
                    fp[mt][:],
                    mybir.ActivationFunctionType.Relu,
                    bias=b1t[:, mt : mt + 1],
                )
                a2t[mt] = a2

            # fc2 feature-major: weights stationary, batch streams; softmax
            # needs batch on partitions, so PE-transpose 128-wide slices.
            p2f = fpsum.tile([FC2_OUT, nb], f32, tag="fps", name=f"p2f_{bt}")
            for mt in range(MT):
                nc.tensor.matmul(
                    p2f[:],
                    w2t[:, mt * FC2_OUT : (mt + 1) * FC2_OUT],
                    a2t[mt][:],
                    start=(mt == 0),
                    stop=(mt == MT - 1),
                )
            s2 = smpool.tile([FC2_OUT, nb], f32, tag="s2")
            nc.scalar.activation(
                s2[:], p2f[:], mybir.ActivationFunctionType.Identity,
                bias=b2t[:, 0:1],
            )
            sub = min(128, nb)
            for s in range(nb // sub):
                tp = fpsum.tile([sub, FC2_OUT], f32, tag="fps",
                                name=f"tp_{bt}_{s}")
                nc.tensor.transpose(
                    tp[:], s2[:, s * sub : (s + 1) * sub], ident[:]
                )
                e = smpool.tile([sub, FC2_OUT], f32, tag="e")
                ssum = smpool.tile([sub, 1], f32, tag="ss")
                nc.scalar.activation(
                    e[:], tp[:], mybir.ActivationFunctionType.Exp,
                    accum_out=ssum[:],
                )
                rinv = smpool.tile([sub, 1], f32, tag="ri")
                nc.vector.reciprocal(rinv[:], ssum[:])
                ot = smpool.tile([sub, FC2_OUT], f32, tag="ot")
                nc.vector.tensor_scalar_mul(ot[:], e[:], rinv[:])
                nc.sync.dma_start(o_d[b0 + s * sub : b0 + (s + 1) * sub, :], ot[:])

    nc.compile()
    return nc


def _prep_weights(conv_w, conv_b, fc1_w, fc1_b, fc2_w, fc2_b):
    conv_w = np.asarray(conv_w, np.float32).reshape(COUT, KS, KS)
    conv_b = np.asarray(conv_b, np.float32)
    fc1_w = np.asarray(fc1_w, np.float32)
    fc1_b = np.asarray(fc1_b, np.float32)
    fc2_w = np.asarray(fc2_w, np.float32)
    fc2_b = np.asarray(fc2_b, np.float32)

    # Toeplitz conv matrix [80, 240]: row = di*16 + jjp (input row offset,
    # input col within 16-wide block); col m = eo*120 + c*6 + q for output
    # col jj = 2q + eo within the 12-wide block.
    T = np.zeros((80, 240), np.float32)
    for m in range(240):
        eo, c, q = m // 120, (m % 120) // 6, m % 6
        jj = 2 * q + eo
        for di in range(KS):
            for dj in range(KS):
                T[di * 16 + jj + dj, m] = conv_w[c, di, dj]

    # fc1 weights permuted to our pooled-feature order:
    # block kb = ip*2 + jb, within-block m = c*6 + q
    # -> original flat feature c*144 + ip*12 + jb*6 + q
    kbv = np.arange(KB)
    ipv, jbv = kbv // 2, kbv % 2
    ml = np.arange(KBS)
    cv, qv = ml // 6, ml % 6
    fidx = cv[None, :] * 144 + ipv[:, None] * 12 + jbv[:, None] * 6 + qv[None, :]
    w1 = np.ascontiguousarray(
        fc1_w.T[fidx.reshape(-1)].reshape(KB, KBS, FC1_OUT)
    )

    # conv bias folded into fc1 bias (pool-max commutes with per-channel const)
    cb_vec = np.repeat(conv_b, 144)
    b1p = fc1_b + fc1_w @ cb_vec
    b1 = np.ascontiguousarray(b1p.reshape(MT, MTS).T)

    w2 = np.ascontiguousarray(
        fc2_w.T.reshape(MT, MTS, FC2_OUT).transpose(1, 0, 2)
    ).reshape(MTS, MT * FC2_OUT)
    b2 = np.ascontiguousarray(fc2_b.reshape(FC2_OUT, 1))
    return T, w1, b1, w2, b2


# im2col pixel indices: idx[jb, di*16+jjp, r] = (r+di)*28 + 12*jb + jjp
_IDX = np.zeros((2, 80, CONV_W_OUT), np.int64)
for _jb in range(2):
    for _di in range(KS):
        for _jjp in range(16):
            for _r in range(CONV_W_OUT):
                _IDX[_jb, _di * 16 + _jjp, _r] = (_r + _di) * W + 12 * _jb + _jjp


def _prep_x(x_core):
    """x_core [784, npc] pixel-major -> xg [2, 80, 24, npc]."""
    return np.ascontiguousarray(x_core[_IDX.reshape(-1)].reshape(
        2, 80, CONV_W_OUT, x_core.shape[1]))


def _run(inputs, npc=NPC, nb=512, trace=False):
    from concourse import bass_utils

    key = (npc, nb)
    if key not in _cache:
        _cache[key] = _build(npc, nb)
    nc = _cache[key]

    T, w1, b1, w2, b2 = _prep_weights(
        inputs["conv_w"], inputs["conv_b"], inputs["fc1_w"],
        inputs["fc1_b"], inputs["fc2_w"], inputs["fc2_b"],
    )
    x = np.asarray(inputs["x"], np.float32).reshape(-1, H * W)
    n_total = x.shape[0]
    assert n_total == NCORES * npc
    xs = x.reshape(NCORES, npc, H * W).transpose(0, 2, 1)

    in_maps = [
        {"xg": _prep_x(xs[i]), "tmat": T, "w1": w1, "b1": b1, "w2": w2,
         "b2": b2}
        for i in range(NCORES)
    ]
    res = bass_utils.run_bass_kernel_spmd(
        nc, in_maps, core_ids=list(range(NCORES)), trace=trace
    )
    out = np.concatenate([res.results[i]["out"] for i in range(NCORES)], axis=0)
    return out, res


def kernel(**inputs):
    out, _ = _run(inputs)
    return out
